# revision 2
# baseline (speedup 1.0000x reference)
"""Dark-channel loss kernel for Trainium2 (8 NeuronCores, batch-parallel).

reference: loss = mean(|MaxPool3d((3,35,35), stride 1, pad (0,17,17))(1 - img)|)
         = 1 - mean(minpool_{3ch,35x35}(img))        (img in [0,1))

Per-core shard: 4 images [3,512,512] fp32. Pipeline per image:
  1. gpsimd cast-DMA loads each channel h-chunk [128,512] fp32->bf16
  2. DVE 2x tensor_tensor min -> channel-min, written into a padded
     [128,560] row tile (1.0 pads; 560 = 16*35 van Herk blocks)
  3. W-direction sliding min-35 by van Herk: segmented fwd/bwd min-scans
     (tensor_tensor_scan, add-BIG mask resets) + combine min
  4. PE transpose (128x128 blocks, bf16, via identity matmul) -> PSUM,
     ACT copies into padded [128,560] column tiles
  5. H-direction sliding min-35: same scans + combine
  6. PE ones-matmul accumulates partition sums of every minHW tile into
     one PSUM [1,512] fp32; DVE reduce -> scalar partial sum per core
Host: loss = 1 - sum(partials) / (N*H*W).

bf16 is safe: min commutes with monotone rounding, so the computed minima
are exactly bf16-rounded true minima; |loss error| <= 2^-9 * mean(min) ~ 5e-7.
"""

import numpy as np

N_CORES = 8
N, C, H, W = 32, 3, 512, 512
PER = N // N_CORES          # images per core
P = 128
WIN = 35
PADL, PADR = 17, 31
L = PADL + 512 + PADR       # 560 = 16 * 35
BIG = 1e9

_cached_nc = None


def _build_nc():
    import concourse.bacc as bacc
    import concourse.mybir as mybir
    from concourse.tile import TileContext
    from concourse.masks import make_identity

    dt = mybir.dt
    Alu = mybir.AluOpType

    nc = bacc.Bacc("TRN2")
    img = nc.declare_dram_parameter("img", [PER, C, H, W], dt.float32,
                                    isOutput=False)
    out = nc.declare_dram_parameter("out", [1, 1], dt.float32, isOutput=True)

    with TileContext(nc) as tc:
        with (
            tc.tile_pool(name="consts", bufs=1) as consts,
            tc.tile_pool(name="ch", bufs=3) as chp,
            tc.tile_pool(name="sr", bufs=3) as srp,
            tc.tile_pool(name="mw", bufs=10) as mwp,
            tc.tile_pool(name="psT", bufs=4, space="PSUM") as psT,
            tc.tile_pool(name="psS", bufs=1, space="PSUM") as psS,
        ):
            # --- constants ---
            mask_f = consts.tile([P, L], dt.bfloat16, tag="mask_f")
            mask_b = consts.tile([P, L], dt.bfloat16, tag="mask_b")
            nc.vector.memset(mask_f[:], 0.0)
            nc.vector.memset(mask_b[:], 0.0)
            mf3 = mask_f[:].rearrange("p (nb w) -> p nb w", w=WIN)
            mb3 = mask_b[:].rearrange("p (nb w) -> p nb w", w=WIN)
            nc.vector.memset(mf3[:, :, 0:1], BIG)
            nc.vector.memset(mb3[:, :, WIN - 1:WIN], BIG)
            ident = consts.tile([P, P], dt.bfloat16, tag="ident")
            make_identity(nc, ident[:])
            ones = consts.tile([P, 1], dt.bfloat16, tag="ones")
            nc.vector.memset(ones[:], 1.0)

            # persistent padded tiles; pads written once, data region cycles
            RW, RU = 4, 8
            wring = [consts.tile([P, L], dt.bfloat16, tag=f"wt{i}", name=f"wt{i}")
                     for i in range(RW)]
            uring = [consts.tile([P, L], dt.bfloat16, tag=f"ut{i}", name=f"ut{i}")
                     for i in range(RU)]
            for t in wring + uring:
                nc.vector.memset(t[:, 0:PADL], 1.0)
                nc.vector.memset(t[:, PADL + 512:L], 1.0)

            cs = psS.tile([1, 512], dt.float32, tag="csum")
            n_mm = PER * 4
            mm_i = 0
            wi = 0

            def slide_min(src, dst):
                """van Herk sliding min-35 along free dim: src [P,560] padded
                -> dst [P,512]."""
                s = srp.tile([P, L], dt.bfloat16, tag="s")
                r = srp.tile([P, L], dt.bfloat16, tag="r")
                nc.vector.tensor_tensor_scan(
                    out=s[:], data0=mask_f[:], data1=src[:], initial=BIG,
                    op0=Alu.add, op1=Alu.min)
                nc.vector.tensor_tensor_scan(
                    out=r[:, ::-1], data0=mask_b[:, ::-1], data1=src[:, ::-1],
                    initial=BIG, op0=Alu.add, op1=Alu.min)
                nc.vector.tensor_tensor(
                    out=dst[:], in0=r[:, 0:512], in1=s[:, WIN - 1:WIN - 1 + 512],
                    op=Alu.min)

            for n in range(PER):
                minw = []
                for c in range(4):          # h-chunks of 128 rows
                    ch = [chp.tile([P, 512], dt.bfloat16, tag=f"ch{k}", name=f"ch{k}_{n}_{c}")
                          for k in range(C)]
                    for k in range(C):
                        nc.gpsimd.dma_start(
                            out=ch[k][:],
                            in_=img[n, k, P * c:P * (c + 1), :])
                    m01 = chp.tile([P, 512], dt.bfloat16, tag="m01")
                    nc.vector.tensor_tensor(out=m01[:], in0=ch[0][:],
                                            in1=ch[1][:], op=Alu.min)
                    wt = wring[wi % RW]
                    wi += 1
                    nc.vector.tensor_tensor(out=wt[:, PADL:PADL + 512],
                                            in0=m01[:], in1=ch[2][:],
                                            op=Alu.min)
                    mw = mwp.tile([P, 512], dt.bfloat16, tag="mw")
                    slide_min(wt, mw)
                    minw.append(mw)

                for w in range(4):          # w-chunks of 128 cols
                    pt = psT.tile([P, 512], dt.bfloat16, tag="pt")
                    for c in range(4):
                        nc.tensor.transpose(pt[:, P * c:P * (c + 1)],
                                            minw[c][:, P * w:P * (w + 1)],
                                            ident[:])
                    ut = uring[(4 * n + w) % RU]
                    nc.scalar.copy(out=ut[:, PADL:PADL + 512], in_=pt[:])
                    mhw = mwp.tile([P, 512], dt.bfloat16, tag="mhw")
                    slide_min(ut, mhw)
                    nc.tensor.matmul(cs[:], ones[:], mhw[:],
                                     start=(mm_i == 0),
                                     stop=(mm_i == n_mm - 1),
                                     skip_group_check=True)
                    mm_i += 1

            tot = consts.tile([1, 1], dt.float32, tag="tot")
            nc.vector.tensor_reduce(out=tot[:], in_=cs[:],
                                    axis=mybir.AxisListType.X, op=Alu.add)
            nc.sync.dma_start(out=out[:], in_=tot[:])

    nc.compile()
    return nc


def _get_nc():
    global _cached_nc
    if _cached_nc is None:
        _cached_nc = _build_nc()
    return _cached_nc


def _partial_sums(img_np):
    from concourse.bass_utils import run_bass_kernel_spmd
    shards = img_np.reshape(N_CORES, PER, C, H, W)
    in_maps = [{"img": np.ascontiguousarray(shards[i])}
               for i in range(N_CORES)]
    res = run_bass_kernel_spmd(_get_nc(), in_maps, list(range(N_CORES)))
    return np.array([float(res.results[i]["out"][0, 0])
                     for i in range(N_CORES)])


def kernel(img):
    img_np = np.asarray(img, dtype=np.float32)
    assert img_np.shape == (N, C, H, W), img_np.shape
    partials = _partial_sums(img_np)
    loss = 1.0 - float(np.sum(partials, dtype=np.float64)) / (N * H * W)
    return np.asarray(loss, dtype=np.float32)


# revision 3
# speedup vs baseline: 1.3520x; 1.3520x over previous
"""Dark-channel loss kernel for Trainium2 (8 NeuronCores, batch-parallel).

reference: loss = mean(|MaxPool3d((3,35,35), stride 1, pad (0,17,17))(1 - img)|)
         = 1 - mean(minpool_{3ch,35x35}(img))        (img in [0,1))

Per-core shard: 4 images [3,512,512] fp32. Pipeline per image:
  1. gpsimd cast-DMA loads each channel h-chunk [128,512] fp32->bf16
  2. DVE 2x tensor_tensor min -> channel-min, written into a padded
     [128,560] row tile (1.0 pads; 560 = 16*35 van Herk blocks)
  3. W-direction sliding min-35 by van Herk: segmented fwd/bwd min-scans
     (tensor_tensor_scan, add-BIG mask resets) + combine min
  4. PE transpose (128x128 blocks, bf16, via identity matmul) -> PSUM,
     ACT copies into padded [128,560] column tiles
  5. H-direction sliding min-35: same scans + combine
  6. PE ones-matmul accumulates partition sums of every minHW tile into
     one PSUM [1,512] fp32; DVE reduce -> scalar partial sum per core
Host: loss = 1 - sum(partials) / (N*H*W).

bf16 is safe: min commutes with monotone rounding, so the computed minima
are exactly bf16-rounded true minima; |loss error| <= 2^-9 * mean(min) ~ 5e-7.
"""

import os
import numpy as np

HWDGE_LOADS = os.environ.get("DC_HWDGE", "0") == "1"

N_CORES = 8
N, C, H, W = 32, 3, 512, 512
PER = N // N_CORES          # images per core
P = 128
WIN = 35
PADL, PADR = 17, 31
L = PADL + 512 + PADR       # 560 = 16 * 35
BIG = 1e9

_cached_nc = None


def _build_nc():
    import concourse.bacc as bacc
    import concourse.mybir as mybir
    from concourse.tile import TileContext
    from concourse.masks import make_identity

    dt = mybir.dt
    Alu = mybir.AluOpType

    nc = bacc.Bacc("TRN2")
    img = nc.declare_dram_parameter("img", [PER, C, H, W], dt.float32,
                                    isOutput=False)
    out = nc.declare_dram_parameter("out", [1, 1], dt.float32, isOutput=True)

    with TileContext(nc) as tc:
        with (
            tc.tile_pool(name="consts", bufs=1) as consts,
            tc.tile_pool(name="ch", bufs=3) as chp,
            tc.tile_pool(name="sr", bufs=3) as srp,
            tc.tile_pool(name="mw", bufs=10) as mwp,
            tc.tile_pool(name="psT", bufs=4, space="PSUM") as psT,
            tc.tile_pool(name="psS", bufs=1, space="PSUM") as psS,
        ):
            # --- constants ---
            mask_f = consts.tile([P, L], dt.bfloat16, tag="mask_f")
            mask_b = consts.tile([P, L], dt.bfloat16, tag="mask_b")
            nc.gpsimd.memset(mask_f[:], 0.0)
            nc.gpsimd.memset(mask_b[:], 0.0)
            mf3 = mask_f[:].rearrange("p (nb w) -> p nb w", w=WIN)
            mb3 = mask_b[:].rearrange("p (nb w) -> p nb w", w=WIN)
            nc.gpsimd.memset(mf3[:, :, 0:1], BIG)
            nc.gpsimd.memset(mb3[:, :, WIN - 1:WIN], BIG)
            ident = consts.tile([P, P], dt.bfloat16, tag="ident")
            make_identity(nc, ident[:])
            ones = consts.tile([P, 1], dt.bfloat16, tag="ones")
            nc.gpsimd.memset(ones[:], 1.0)

            # persistent padded tiles; pads written once, data region cycles
            RW, RU = 4, 8
            wring = [consts.tile([P, L], dt.bfloat16, tag=f"wt{i}", name=f"wt{i}")
                     for i in range(RW)]
            uring = [consts.tile([P, L], dt.bfloat16, tag=f"ut{i}", name=f"ut{i}")
                     for i in range(RU)]
            for t in wring + uring:
                nc.gpsimd.memset(t[:, 0:PADL], 1.0)
                nc.gpsimd.memset(t[:, PADL + 512:L], 1.0)

            cs = psS.tile([1, 512], dt.float32, tag="csum")
            n_mm = PER * 4
            mm_i = 0
            wi = 0

            def slide_min(src, dst):
                """van Herk sliding min-35 along free dim: src [P,560] padded
                -> dst [P,512]."""
                s = srp.tile([P, L], dt.bfloat16, tag="s")
                r = srp.tile([P, L], dt.bfloat16, tag="r")
                nc.vector.tensor_tensor_scan(
                    out=s[:], data0=mask_f[:], data1=src[:], initial=BIG,
                    op0=Alu.add, op1=Alu.min)
                nc.vector.tensor_tensor_scan(
                    out=r[:, ::-1], data0=mask_b[:, ::-1], data1=src[:, ::-1],
                    initial=BIG, op0=Alu.add, op1=Alu.min)
                nc.vector.tensor_tensor(
                    out=dst[:], in0=r[:, 0:512], in1=s[:, WIN - 1:WIN - 1 + 512],
                    op=Alu.min)

            for n in range(PER):
                minw = []
                for c in range(4):          # h-chunks of 128 rows
                    ch = [chp.tile([P, 512], dt.bfloat16, tag=f"ch{k}", name=f"ch{k}_{n}_{c}")
                          for k in range(C)]
                    if HWDGE_LOADS:
                        for k in range(C):
                            chf = chp.tile([P, 512], dt.float32,
                                           tag=f"chf{k}", name=f"chf{k}_{n}_{c}")
                            nc.sync.dma_start(
                                out=chf[:],
                                in_=img[n, k, P * c:P * (c + 1), :])
                            nc.scalar.copy(out=ch[k][:], in_=chf[:])
                    else:
                        for k in range(C):
                            nc.gpsimd.dma_start(
                                out=ch[k][:],
                                in_=img[n, k, P * c:P * (c + 1), :])
                    m01 = chp.tile([P, 512], dt.bfloat16, tag="m01")
                    nc.vector.tensor_tensor(out=m01[:], in0=ch[0][:],
                                            in1=ch[1][:], op=Alu.min)
                    wt = wring[wi % RW]
                    wi += 1
                    nc.vector.tensor_tensor(out=wt[:, PADL:PADL + 512],
                                            in0=m01[:], in1=ch[2][:],
                                            op=Alu.min)
                    mw = mwp.tile([P, 512], dt.bfloat16, tag="mw")
                    slide_min(wt, mw)
                    minw.append(mw)

                for w in range(4):          # w-chunks of 128 cols
                    pt = psT.tile([P, 512], dt.bfloat16, tag="pt")
                    for c in range(4):
                        nc.tensor.transpose(pt[:, P * c:P * (c + 1)],
                                            minw[c][:, P * w:P * (w + 1)],
                                            ident[:])
                    ut = uring[(4 * n + w) % RU]
                    nc.scalar.copy(out=ut[:, PADL:PADL + 512], in_=pt[:])
                    mhw = mwp.tile([P, 512], dt.bfloat16, tag="mhw")
                    slide_min(ut, mhw)
                    nc.tensor.matmul(cs[:], ones[:], mhw[:],
                                     start=(mm_i == 0),
                                     stop=(mm_i == n_mm - 1),
                                     skip_group_check=True)
                    mm_i += 1

            tot = consts.tile([1, 1], dt.float32, tag="tot")
            nc.vector.tensor_reduce(out=tot[:], in_=cs[:],
                                    axis=mybir.AxisListType.X, op=Alu.add)
            nc.sync.dma_start(out=out[:], in_=tot[:])

    nc.compile()
    return nc


def _get_nc():
    global _cached_nc
    if _cached_nc is None:
        _cached_nc = _build_nc()
    return _cached_nc


def _partial_sums(img_np):
    from concourse.bass_utils import run_bass_kernel_spmd
    shards = img_np.reshape(N_CORES, PER, C, H, W)
    in_maps = [{"img": np.ascontiguousarray(shards[i])}
               for i in range(N_CORES)]
    res = run_bass_kernel_spmd(_get_nc(), in_maps, list(range(N_CORES)))
    return np.array([float(res.results[i]["out"][0, 0])
                     for i in range(N_CORES)])


def kernel(img):
    img_np = np.asarray(img, dtype=np.float32)
    assert img_np.shape == (N, C, H, W), img_np.shape
    partials = _partial_sums(img_np)
    loss = 1.0 - float(np.sum(partials, dtype=np.float64)) / (N * H * W)
    return np.asarray(loss, dtype=np.float32)
